# revision 44
# baseline (speedup 1.0000x reference)
"""Trainium2 Bass kernel for nn_CrossAttention (single-query cross attention).

Reference computation (B=4, C=64, H=W=128, heads h=64, dim_head d=64,
inner=4096, HW=16384):
    x[b, j, c]   = fimg[b, c, j]                       (j indexes H*W)
    q[b, h, d]   = sum_e fpsf[b, e] Wq[h*64+d, e]
    k[b, j, h, d]= sum_c x[b, j, c] Wk[h*64+d, c]
    out[b, h, j] = scale * sum_d q[b,h,d] k[b,j,h,d]

Because there is a single query per (batch, head), the attention collapses:
    W2[b, h, c]  = scale * sum_d q[b,h,d] Wk[h*64+d, c]      (tiny)
    out[b, h, j] = sum_c W2[b,h,c] fimg[b, c, j]
a 64x FLOP reduction vs materializing k.

Sharding: the j (H*W = 16384) axis is split across the 8 cores (2048 each).
Every core redundantly computes W2 (it needs all heads for its output).

v14 (raw bass, no TileContext, fully balanced DMA):
  - SDMA engine k serves a fixed set of 8 SBUF partitions, so a [64, N]
    transfer only engages half the 16 engines (~185 GB/s vs ~370).
    All four transfers here are [128, N]: Wq and the dense Wk carry
    their second half on partitions 64:127 and DVE/ACT lower those rows
    to base-0 tiles (matmul operands at partition offset 64 hard-crash
    the PE - verified). Total input drops 2.5MB -> 2.0MB per core and
    every byte moves at full engine rate.
  - Hand-rolled semaphores (~18) instead of the tile framework's
    scheduler sync; input DMAs in priority order Wq, Wk, fimg.
  - fimg lands in four 1024-col chunks with per-chunk semaphores: a DMA
    completes only when all 16 engines finish and 1-2 straggler engines
    lag ~2us, so the big matmuls gate per-chunk instead of on the whole
    tensor; the last output DMA is split the same way.
  - Output staged and DMA'd as bf16 (host casts back to f32).

Device layouts (prepared host-side; host does LAYOUT only, no math):
  Wq2    [128, 2052] bf16: rows 0:64  = [fpsf.T | Wq.T cols 0:2048]
                           rows 64:128= [0      | Wq.T cols 2048:4096]
  Wk2    [128, 2048] bf16: with Wk_nat[d, 64h+c] = Wk[64h+d, c]:
                           rows 0:64 = Wk_nat cols 0:2048 (pairs 0-15),
                           rows 64:128 = cols 2048:4096 (pairs 16-31)
  fimg_s [128, 4096] bf16: rows b%2*64+c, cols 2048*(b//2) + local j
  out    [128, 4096] bf16: rows b%2*64+h, cols 2048*(b//2) + local j

Device compute per core:
  lower: wq_hi[64, 2048] <- Wq2[64:128, 4:], wk_hi <- Wk2[64:128, :]
     (halves on DVE and ACT in parallel)
  A: 32 matmuls (16 lo + 16 hi): q2T chunk [128, 4] = WqT_chunk.T@fpsfT
     -> q2T psum [128, 128]: rows d+64*(h%2), cols 4*(h//2)+b
  copy: psum halves -> SBUF bf16 q2e/q2o [64, 128] (scale folded)
  B: 64 matmuls; head-pair p loads its [64, 128] Wk pair block (128-col
     stationary -> compiler FWL) twice: rhs q2e[:, 4p:4p+4] ->
     w2a[c, 4p+b] (rows 0:64 valid), rhs q2o -> w2b (rows 64:128 valid)
  Assembly: per batch-pair q, block-diag lhsT bd_q [128, 128] bf16:
     bd_q[64*half + c, 64*half + h] = W2[2q+half, h, c]
  Big: 8 matmuls [128, 512] = bd_q.T @ fimg cols; psum -> bf16 staging
     (vector/scalar alternate); one out DMA per q on the Sync ring.
"""

import sys
import types

import numpy as np
import ml_dtypes

# antenv.axon_hooks is absent in this image; bass_utils imports it when
# tracing. Register a minimal stand-in before importing concourse.
if "antenv.axon_hooks" not in sys.modules:
    try:
        import antenv  # noqa: F401

        _hooks = types.ModuleType("antenv.axon_hooks")
        _hooks._hook = None

        def _set_hook(h):
            _hooks._hook = h

        _hooks.set_axon_ntff_profile_hook = _set_hook
        _hooks.get_axon_ntff_profile_hook = lambda: _hooks._hook
        sys.modules["antenv.axon_hooks"] = _hooks
        try:
            from trn_agent_boot.trn_boot import _ntff_profile_via_ctypes

            _set_hook(_ntff_profile_via_ctypes("/opt/axon/libaxon_pjrt.so"))
        except Exception:
            pass
    except ImportError:
        pass

import concourse.bass as bass  # noqa: E402
import concourse.mybir as mybir  # noqa: E402
import concourse.tile as tile  # noqa: E402
from concourse import bacc  # noqa: E402
from concourse.bass_utils import run_bass_kernel_spmd  # noqa: E402

N_CORES = 8
B, C, H, W = 4, 64, 128, 128
HEADS, DIM_HEAD = 64, 64
HW = H * W
JS = HW // N_CORES  # 2048 j-positions per core
SCALE = DIM_HEAD ** -0.5
F32 = mybir.dt.float32
BF16 = mybir.dt.bfloat16
NPBF16 = ml_dtypes.bfloat16

_compiled = None  # cache (nc) across calls


def _build():
    nc = bacc.Bacc("TRN2", target_bir_lowering=False, debug=False,
                   num_devices=N_CORES)

    fimg_d = nc.dram_tensor("fimg_s", [128, 2 * JS], BF16, kind="ExternalInput")
    wq2_d = nc.dram_tensor("Wq2", [128, 2052], BF16, kind="ExternalInput")
    wk2_d = nc.dram_tensor("Wk2", [128, 2048], BF16, kind="ExternalInput")
    out_d = nc.dram_tensor("out", [128, 2 * JS], BF16, kind="ExternalOutput")

    # SBUF
    wq2 = nc.alloc_sbuf_tensor("wq2", [128, 2052], BF16)
    wk2 = nc.alloc_sbuf_tensor("wk2", [128, 2048], BF16)
    wq_hi = nc.alloc_sbuf_tensor("wq_hi", [64, 2048], BF16)
    wk_hi = nc.alloc_sbuf_tensor("wk_hi", [64, 2048], BF16)
    imgs = nc.alloc_sbuf_tensor("imgs", [128, 2 * JS], BF16)
    q2e = nc.alloc_sbuf_tensor("q2e", [64, 128], BF16)
    q2o = nc.alloc_sbuf_tensor("q2o", [64, 128], BF16)
    bd0 = nc.alloc_sbuf_tensor("bd0", [128, 128], BF16)
    bd1 = nc.alloc_sbuf_tensor("bd1", [128, 128], BF16)
    ot0 = nc.alloc_sbuf_tensor("ot0", [128, JS], BF16)
    ot1 = nc.alloc_sbuf_tensor("ot1", [128, JS], BF16)
    fpsfT = wq2[0:64, 0:4]

    # PSUM: 2 small banks + 6 rotating banks for the 8 big matmuls
    q2T_ps = nc.alloc_psum_tensor("q2T_ps", [128, 128], F32)
    w2ab = nc.alloc_psum_tensor("w2ab", [128, 256], F32)
    w2a = w2ab[:, 0:128]
    w2b = w2ab[:, 128:256]
    big_ps = [nc.alloc_psum_tensor(f"big{i}", [128, 512], F32)
              for i in range(6)]

    # Semaphores (contiguous range for the teardown range-clear)
    sWq = nc.alloc_semaphore("sWq")    # Wq2 in
    sWk = nc.alloc_semaphore("sWk")    # Wk2 in
    sD = [nc.alloc_semaphore(f"sD{c}") for c in range(4)]  # fimg chunks
    sQH = nc.alloc_semaphore("sQH")    # wq_hi lowered (2 halves)
    sKH = nc.alloc_semaphore("sKH")    # wk_hi lowered (2 halves)
    sT1 = nc.alloc_semaphore("sT1")    # A done
    sVq = nc.alloc_semaphore("sVq")    # q2e copy done
    sSq = nc.alloc_semaphore("sSq")    # q2o copy done
    sT2 = nc.alloc_semaphore("sT2")    # B done
    sV2a = nc.alloc_semaphore("sV2a")  # bd0 assembled
    sV2b = nc.alloc_semaphore("sV2b")  # bd1 assembled
    sT3 = nc.alloc_semaphore("sT3")    # big matmul counter
    sCv = nc.alloc_semaphore("sCv")    # vector casts done
    sCs = nc.alloc_semaphore("sCs")    # scalar casts done
    sO = nc.alloc_semaphore("sO")      # out DMAs done
    sems = [sWq, sWk, *sD, sQH, sKH, sT1, sVq, sSq, sT2, sV2a, sV2b,
            sT3, sCv, sCs, sO]
    sem_range = range(min(s.num for s in sems), max(s.num for s in sems) + 1)
    assert len(sem_range) == len(sems), (sem_range, [s.num for s in sems])

    # --- Sync engine: input DMAs in priority order, then output DMAs ---
    nc.sync.dma_start(wq2[:], wq2_d.ap()[:]).then_inc(sWq, 16)
    nc.sync.dma_start(wk2[:], wk2_d.ap()[:]).then_inc(sWk, 16)
    for c in range(2):
        nc.sync.dma_start(imgs[:, 1024 * c:1024 * c + 1024],
                          fimg_d.ap()[:, 1024 * c:1024 * c + 1024]
                          ).then_inc(sD[c], 16)
    # Issue the last two fimg chunks only after the weight semaphore:
    # sem delivery on a ring stalls until its issue stream completes, so
    # a short initial stream lets the weight gates fire ~1.5us earlier.
    # c2/c3 descriptors are still written well before their data window.
    nc.sync.wait_ge(sWq, 16)
    for c in (2, 3):
        nc.sync.dma_start(imgs[:, 1024 * c:1024 * c + 1024],
                          fimg_d.ap()[:, 1024 * c:1024 * c + 1024]
                          ).then_inc(sD[c], 16)
    nc.sync.wait_ge(sCv, 2)
    nc.sync.wait_ge(sCs, 2)
    nc.sync.dma_start(out_d.ap()[:, 0:JS], ot0[:]).then_inc(sO, 16)
    nc.sync.wait_ge(sCv, 3)
    nc.sync.wait_ge(sCs, 3)
    nc.sync.dma_start(out_d.ap()[:, JS:JS + 1024],
                      ot1[:, 0:1024]).then_inc(sO, 16)
    # ot1[1024:1536] needs only the Vector cast of big matmul 6, which
    # retires ~0.4us before the Scalar cast of matmul 7 - issue it early
    # so the final gated transfer is only 512 cols.
    nc.sync.wait_ge(sCv, 4)
    nc.sync.dma_start(out_d.ap()[:, JS + 1024:JS + 1536],
                      ot1[:, 1024:1536]).then_inc(sO, 16)
    nc.sync.wait_ge(sCs, 4)
    nc.sync.dma_start(out_d.ap()[:, JS + 1536:2 * JS],
                      ot1[:, 1536:2048]).then_inc(sO, 16)
    nc.sync.wait_ge(sO, 64)

    # --- Tensor engine ---
    # A: q2T[d + 64*(h%2), 4p+b] = q[b, 2p + (row>=64), d]
    nc.tensor.wait_ge(sWq, 16)
    for p in range(16):
        nc.tensor.matmul(q2T_ps[:, 4 * p:4 * p + 4],
                         wq2[0:64, 4 + 128 * p:4 + 128 * p + 128], fpsfT,
                         start=True, stop=True)
    nc.tensor.wait_ge(sQH, 2)
    for pp in range(16):
        mm = nc.tensor.matmul(q2T_ps[:, 64 + 4 * pp:64 + 4 * pp + 4],
                              wq_hi[:, 128 * pp:128 * pp + 128], fpsfT,
                              start=True, stop=True)
        if pp == 15:
            mm.then_inc(sT1, 1)
    # B: per pair p, the [64, 128] stationary is loaded twice:
    #   w2a[c, 4p+b]    = W2[b, 2p, c]/scale   (rows 64:128 garbage)
    #   w2b[64+c, 4p+b] = W2[b, 2p+1, c]/scale (rows 0:64 garbage)
    nc.tensor.wait_ge(sVq, 1)
    nc.tensor.wait_ge(sSq, 1)
    nc.tensor.wait_ge(sWk, 16)
    for p in range(32):
        if p == 16:
            nc.tensor.wait_ge(sKH, 2)
        if p < 16:
            lhsT = wk2[0:64, 128 * p:128 * p + 128]
        else:
            lhsT = wk_hi[:, 128 * (p - 16):128 * (p - 16) + 128]
        nc.tensor.matmul(w2a[:, 4 * p:4 * p + 4], lhsT,
                         q2e[:, 4 * p:4 * p + 4], start=True, stop=True)
        mm = nc.tensor.matmul(w2b[:, 4 * p:4 * p + 4], lhsT,
                              q2o[:, 4 * p:4 * p + 4], start=True, stop=True)
        if p == 31:
            mm.then_inc(sT2, 1)
    # Big: out rows pair q = bd_q.T @ img_q, 512-col chunks, 6-bank rotation
    nc.tensor.wait_ge(sV2a, 1)
    for i in range(8):
        q, k = divmod(i, 4)
        if i % 2 == 0:
            nc.tensor.wait_ge(sD[i // 2], 16)
        if i == 4:
            nc.tensor.wait_ge(sV2b, 1)
        if i == 6:
            nc.tensor.wait_ge(sCv, 1)   # bank big0's cast (V) retired
        if i == 7:
            nc.tensor.wait_ge(sCs, 1)   # bank big1's cast (S) retired
        bd = bd0 if q == 0 else bd1
        nc.tensor.matmul(big_ps[i % 6][:], bd[:],
                         imgs[:, JS * q + 512 * k:JS * q + 512 * k + 512],
                         start=True, stop=True).then_inc(sT3, 1)

    # --- Vector engine ---
    nc.vector.memset(bd0[:], 0.0)
    nc.vector.memset(bd1[:], 0.0)
    nc.vector.wait_ge(sWq, 16)
    nc.vector.tensor_copy(wq_hi[:, 0:1536],
                          wq2[64:128, 4:1540]).then_inc(sQH, 1)
    nc.vector.wait_ge(sT1, 1)
    nc.vector.tensor_scalar_mul(q2e[:], q2T_ps[0:64, :],
                                SCALE).then_inc(sVq, 1)
    nc.vector.wait_ge(sWk, 16)
    nc.vector.tensor_copy(wk_hi[:, 0:1536],
                          wk2[64:128, 0:1536]).then_inc(sKH, 1)
    nc.vector.wait_ge(sT2, 1)
    for half in range(2):
        b = half  # q = 0
        for parity in range(2):
            cp = nc.vector.tensor_copy(
                bd0[64 * half:64 * half + 64,
                    64 * half + parity:64 * half + 64:2],
                (w2a if parity == 0 else w2b)[
                    64 * parity:64 * parity + 64, b:128:4])
    cp.then_inc(sV2a, 1)
    for i in (0, 2, 4, 6):
        q, k = divmod(i, 4)
        ot = ot0 if q == 0 else ot1
        nc.vector.wait_ge(sT3, i + 1)
        nc.vector.tensor_copy(ot[:, 512 * k:512 * k + 512],
                              big_ps[i % 6][:]).then_inc(sCv, 1)

    # --- Scalar engine ---
    nc.scalar.wait_ge(sWq, 16)
    nc.scalar.copy(wq_hi[:, 1536:2048],
                   wq2[64:128, 1540:2052]).then_inc(sQH, 1)
    nc.scalar.wait_ge(sT1, 1)
    nc.scalar.mul(q2o[:], q2T_ps[64:128, :], SCALE).then_inc(sSq, 1)
    nc.scalar.wait_ge(sWk, 16)
    nc.scalar.copy(wk_hi[:, 1536:2048],
                   wk2[64:128, 1536:2048]).then_inc(sKH, 1)
    nc.scalar.wait_ge(sT2, 1)
    for half in range(2):
        b = 2 + half  # q = 1
        for parity in range(2):
            cp = nc.scalar.copy(
                bd1[64 * half:64 * half + 64,
                    64 * half + parity:64 * half + 64:2],
                (w2a if parity == 0 else w2b)[
                    64 * parity:64 * parity + 64, b:128:4])
    cp.then_inc(sV2b, 1)
    for i in (1, 3, 5, 7):
        q, k = divmod(i, 4)
        ot = ot0 if q == 0 else ot1
        nc.scalar.wait_ge(sT3, i + 1)
        nc.scalar.copy(ot[:, 512 * k:512 * k + 512],
                       big_ps[i % 6][:]).then_inc(sCs, 1)

    # --- Teardown: reset semaphores so the NEFF is re-runnable ---
    nc.all_engine_barrier()
    nc.gpsimd.dma_reset(sem_range)
    nc.gpsimd.sem_clear(sem_range)
    nc.all_engine_barrier()

    nc.compile()
    return nc


def _prep_inputs(fpsf, fimg, Wq, Wk):
    fpsf = np.ascontiguousarray(fpsf, dtype=np.float32)
    fimg = np.ascontiguousarray(fimg, dtype=np.float32)
    Wq = np.ascontiguousarray(Wq, dtype=np.float32)
    Wk = np.ascontiguousarray(Wk, dtype=np.float32)

    WqT = Wq.T.astype(NPBF16)  # [64, 4096]
    Wq2 = np.zeros((128, 2052), NPBF16)
    Wq2[0:64, 0:4] = fpsf.T.astype(NPBF16)
    Wq2[0:64, 4:2052] = WqT[:, 0:2048]
    Wq2[64:128, 4:2052] = WqT[:, 2048:4096]

    # Wk_nat[d, 64h+c] = Wk[64h+d, c]
    Wk_nat = np.ascontiguousarray(
        Wk.reshape(64, 64, 64).transpose(1, 0, 2).reshape(64, 4096)
    ).astype(NPBF16)
    Wk2 = np.empty((128, 2048), NPBF16)
    Wk2[0:64, :] = Wk_nat[:, 0:2048]
    Wk2[64:128, :] = Wk_nat[:, 2048:4096]

    fimg_f = fimg.reshape(B, C, HW).astype(NPBF16)
    in_maps = []
    for i in range(N_CORES):
        sh = np.ascontiguousarray(
            fimg_f[:, :, JS * i:JS * (i + 1)]).reshape(2, 128, JS)
        sh = np.ascontiguousarray(
            sh.transpose(1, 0, 2).reshape(128, 2 * JS))
        in_maps.append({
            "fimg_s": sh,
            "Wq2": Wq2,
            "Wk2": Wk2,
        })
    return in_maps


def kernel(fpsf, fimg, Wq, Wk):
    global _compiled
    if _compiled is None:
        _compiled = _build()
    nc = _compiled

    in_maps = _prep_inputs(fpsf, fimg, Wq, Wk)
    res = run_bass_kernel_spmd(nc, in_maps, core_ids=list(range(N_CORES)))

    out = np.empty((B, HEADS, HW), dtype=np.float32)
    for i in range(N_CORES):
        o = res.results[i]["out"]  # [128, 2*JS] bf16
        o = o.reshape(128, 2, JS).transpose(1, 0, 2).reshape(B, HEADS, JS)
        out[:, :, JS * i:JS * (i + 1)] = o.astype(np.float32)
    return out.reshape(B, C, H, W)


if __name__ == "__main__":
    rng = np.random.default_rng(0)
    ins = {
        "fpsf": rng.standard_normal((B, C), dtype=np.float32),
        "fimg": rng.standard_normal((B, C, H, W), dtype=np.float32),
        "Wq": (rng.standard_normal((4096, C), dtype=np.float32) * 0.05),
        "Wk": (rng.standard_normal((4096, C), dtype=np.float32) * 0.05),
    }
    out = kernel(**ins)
    print("out", out.shape, out.dtype, float(np.abs(out).max()))
